# revision 2
# baseline (speedup 1.0000x reference)
"""Distributed Trainium2 Bass kernel v2 for the GAT-Actor (gnn_message_passing).

Changes vs baseline kernel.py:
  - table rows are 256 B (h in f16) instead of 768 B f32: gather traffic 3x lower
  - e_src recomputed per gathered block on DVE (STT accum vs a_src)
  - all matmuls f16 (fp32 ran at 1/4 rate, mostly cold-clocked)
  - agg matmul operand-swapped: lhsT = one-hot S_b -> out [dst, h] in PSUM,
    denominator via a second N=1 matmul with the same weights
  - AllGather split into two half-shard collectives so first-half gathers
    overlap the second collective
  - gather idx streams sorted by src within (chunk, half); pad idxs are -1 at
    stream end (skipped by HW, no traffic)
"""

import os
import sys

for _p in ("/opt/trn_rl_repo", "/root/.axon_site/_ro/trn_rl_repo"):
    if os.path.isdir(_p) and _p not in sys.path:
        sys.path.insert(0, _p)

import numpy as np
import ml_dtypes

from concourse import bass, bacc, tile, mybir
from concourse.bass_utils import run_bass_kernel_spmd

f32 = mybir.dt.float32
f16 = mybir.dt.float16
i16 = mybir.dt.int16
AF = mybir.ActivationFunctionType
ALU = mybir.AluOpType

NCORES = 8
C = 128                # dst chunk width
NEG_SLOPE = 0.2
EPS = 1e-5

_cache = {}
last_results = None


# --------------------------------------------------------------------------
# host-side edge preprocessing
# --------------------------------------------------------------------------

def _wrap_idx(idx):
    idx = np.asarray(idx, np.int16)
    m = idx.shape[0]
    assert m % 16 == 0
    arr = idx.reshape(m // 16, 16).T
    return np.ascontiguousarray(np.tile(arr, (8, 1)))


def _prep_edges(edge_index, N, NLOC):
    """Group edges per core by (dst-chunk, src-half); half = src local offset
    < HLOC (AllGather half-shard split). Within (chunk, half) sort by src.
    Returns per-core tensors + compile-time block structure."""
    HLOC = NLOC // 2           # 3125
    NCH = -(-NLOC // C)
    src = np.asarray(edge_index[0], np.int64)
    dst = np.asarray(edge_index[1], np.int64)

    cores = []
    counts = np.zeros((NCORES, NCH, 2), np.int64)
    for i in range(NCORES):
        sel = (dst // NLOC) == i
        s = src[sel]
        d = dst[sel] - i * NLOC
        ch = d // C
        so = s % NLOC          # local offset within owner core's shard
        hf = (so >= HLOC).astype(np.int64)
        # remapped row index within the half-table [NCORES*HLOC, H]
        rown = (s // NLOC) * HLOC + (so - hf * HLOC)
        order = np.lexsort((rown, hf, ch))
        s, d, ch, hf, rown = s[order], d[order], ch[order], hf[order], rown[order]
        for c in range(NCH):
            for h in range(2):
                counts[i, c, h] = np.count_nonzero((ch == c) & (hf == h))
        cores.append((rown, d, ch, hf))

    # compile-time: blocks and idx-stream lengths per (chunk, half)
    NA = [int(-(-counts[:, c, 0].max() // 128)) for c in range(NCH)]
    NB = [int(-(-counts[:, c, 1].max() // 128)) for c in range(NCH)]
    NIA = [int(-(-counts[:, c, 0].max() // 16) * 16) for c in range(NCH)]
    NIB = [int(-(-counts[:, c, 1].max() // 16) * 16) for c in range(NCH)]
    for c in range(NCH):
        if NA[c] + NB[c] == 0:
            NA[c] = 1
            NIA[c] = 16

    TOTB = sum(NA) + sum(NB)
    TOTI = sum(NIA) + sum(NIB)

    per_core = []
    for i in range(NCORES):
        rown, d, ch, hf = cores[i]
        idx_stream = np.zeros(TOTI, np.int16)
        dst_rel = np.full(TOTB * 128, -1.0, np.float32)
        ipos = 0
        bpos = 0
        ptr = 0
        for c in range(NCH):
            for h, nblk, nidx in ((0, NA[c], NIA[c]), (1, NB[c], NIB[c])):
                cnt = int(counts[i, c, h])
                sl = slice(ptr, ptr + cnt)
                rr, dd = rown[sl], d[sl]
                ptr += cnt
                if cnt:
                    idx_stream[ipos:ipos + cnt] = rr.astype(np.int16)
                    dst_rel[bpos:bpos + cnt] = (dd - c * C).astype(np.float32)
                ipos += nidx
                bpos += nblk * 128
        assert ptr == len(d)
        dst_rel_t = np.ascontiguousarray(dst_rel.reshape(TOTB, 128).T)
        per_core.append({
            "src_idx": _wrap_idx(idx_stream),
            "dst_rel": dst_rel_t,
        })
    return per_core, NA, NB, NIA, NIB


# --------------------------------------------------------------------------
# device graph
# --------------------------------------------------------------------------

def _build_nc(N, D, H, A, NLOC, NA, NB, NIA, NIB):
    KD = D // 128
    NT = -(-NLOC // 128)
    NLOCP = NT * 128           # 6272
    HLOC = NLOC // 2
    NHALF = NCORES * HLOC      # rows per gathered half table
    NCH = len(NA)
    TOTB = sum(NA) + sum(NB)
    TOTI = sum(NIA) + sum(NIB)
    NBFmax = max(NA[c] + NB[c] for c in range(NCH))

    nc = bacc.Bacc("TRN2", num_devices=NCORES)

    # ---- inputs
    xbA = nc.dram_tensor("xbA", [NLOCP, 128], f16, kind="ExternalInput")
    xbB = nc.dram_tensor("xbB", [NLOCP, 128], f16, kind="ExternalInput")
    W_in = nc.dram_tensor("W", [D, H], f32, kind="ExternalInput")
    adst_in = nc.dram_tensor("adst", [128, 1], f32, kind="ExternalInput")
    asrcb = nc.dram_tensor("asrc_b", [128, H], f32, kind="ExternalInput")
    bgat = nc.dram_tensor("b_gat", [H, 1], f32, kind="ExternalInput")
    bn0p = nc.dram_tensor("bn0p", [H, 2], f32, kind="ExternalInput")
    bn2p = nc.dram_tensor("bn2p", [H, 2], f32, kind="ExternalInput")
    W1_in = nc.dram_tensor("W1", [H, H], f32, kind="ExternalInput")
    b1_in = nc.dram_tensor("b1", [H, 1], f32, kind="ExternalInput")
    W2_in = nc.dram_tensor("W2", [H, H], f32, kind="ExternalInput")
    b2_in = nc.dram_tensor("b2", [H, 1], f32, kind="ExternalInput")
    W3_in = nc.dram_tensor("W3", [H, A], f32, kind="ExternalInput")
    b3_in = nc.dram_tensor("b3", [A, 1], f32, kind="ExternalInput")
    ident_in = nc.dram_tensor("ident", [128, 128], f32, kind="ExternalInput")
    iota_in = nc.dram_tensor("iota_b", [128, C], f32, kind="ExternalInput")
    onesrow_in = nc.dram_tensor("ones_row", [1, 128], f32, kind="ExternalInput")
    srci_in = nc.dram_tensor("src_idx", [128, TOTI // 16], i16, kind="ExternalInput")
    dstr_in = nc.dram_tensor("dst_rel", [128, TOTB], f32, kind="ExternalInput")

    out_t = nc.dram_tensor("out", [NLOC, A], f32, kind="ExternalOutput")

    with tile.TileContext(nc) as tc:
        with tc.tile_pool(name="const", bufs=1) as cp, \
             tc.tile_pool(name="dram", bufs=1, space="DRAM") as dram, \
             tc.tile_pool(name="big", bufs=1) as bigp:

            # ---- constants
            ident = cp.tile([128, 128], f32)
            nc.sync.dma_start(ident[:], ident_in[:])
            identb = cp.tile([128, 128], f16)
            nc.vector.tensor_copy(identb[:], ident[:])
            asrc_sb = cp.tile([128, H], f32)
            nc.sync.dma_start(asrc_sb[:], asrcb[:])
            asrc_bf = cp.tile([128, H], f16)
            nc.vector.tensor_copy(asrc_bf[:], asrc_sb[:])
            adst_sb = cp.tile([128, 1], f32)
            nc.sync.dma_start(adst_sb[:], adst_in[:])
            bgat_sb = cp.tile([H, 1], f32)
            nc.sync.dma_start(bgat_sb[:], bgat[:])
            bn0_sb = cp.tile([H, 2], f32)
            nc.sync.dma_start(bn0_sb[:], bn0p[:])
            bn2_sb = cp.tile([H, 2], f32)
            nc.sync.dma_start(bn2_sb[:], bn2p[:])
            W1_sb = cp.tile([H, H], f32)
            nc.sync.dma_start(W1_sb[:], W1_in[:])
            b1_sb = cp.tile([H, 1], f32)
            nc.sync.dma_start(b1_sb[:], b1_in[:])
            W2_sb = cp.tile([H, H], f32)
            nc.sync.dma_start(W2_sb[:], W2_in[:])
            W2_bf = cp.tile([H, H], f16)
            nc.vector.tensor_copy(W2_bf[:], W2_sb[:])
            b2_sb = cp.tile([H, 1], f32)
            nc.sync.dma_start(b2_sb[:], b2_in[:])
            W3_sb = cp.tile([H, A], f32)
            nc.sync.dma_start(W3_sb[:], W3_in[:])
            b3_sb = cp.tile([A, 1], f32)
            nc.sync.dma_start(b3_sb[:], b3_in[:])
            iota_sb = cp.tile([128, C], f32)
            nc.sync.dma_start(iota_sb[:], iota_in[:])
            onesr = cp.tile([1, 128], f32)
            nc.sync.dma_start(onesr[:], onesrow_in[:])
            onesc_bf = cp.tile([128, 1], f16)
            nc.vector.memset(onesc_bf[:], 1.0)
            neg8 = cp.tile([128, 1], f32)
            nc.vector.memset(neg8[:], -8.0)
            srci_sb = bigp.tile([128, TOTI // 16], i16)
            nc.sync.dma_start(srci_sb[:], srci_in[:])
            dstr_sb = bigp.tile([128, TOTB], f32)
            nc.sync.dma_start(dstr_sb[:], dstr_in[:])

            # ---- internal DRAM
            hlocA_d = dram.tile([HLOC, H], f16)
            hlocB_d = dram.tile([HLOC, H], f16)
            hfullA_d = dram.tile([NHALF, H], f16, addr_space="Shared")
            hfullB_d = dram.tile([NHALF, H], f16, addr_space="Shared")
            bn_in_0 = dram.tile([H, 2], f32)
            bn_out_0 = dram.tile([H, 2], f32, addr_space="Shared")
            bn_in_1 = dram.tile([H, 2], f32)
            bn_out_1 = dram.tile([H, 2], f32, addr_space="Shared")

            # ---- load W, build W2cols_k = [W_k | vd_k] in f16
            W_sb = cp.tile([128, KD, H], f32)
            nc.sync.dma_start(W_sb[:], bass.AP(W_in, 0, [[H, 128], [128 * H, KD], [1, H]]))
            W2c = cp.tile([128, KD, H + 1], f16)
            with tc.tile_pool(name="wprep", bufs=2) as wpp, \
                 tc.tile_pool(name="wps", bufs=2, space="PSUM") as wps:
                for k in range(KD):
                    nc.vector.tensor_copy(W2c[:, k, 0:H], W_sb[:, k, :])
                    wT_ps = wps.tile([128, 128], f32, tag="wT")
                    nc.tensor.transpose(wT_ps[:], W_sb[:, k, :], ident[:])
                    wT_sb = wpp.tile([128, 128], f32, tag="wTs")
                    nc.vector.tensor_copy(wT_sb[:], wT_ps[:])
                    vd_ps = wps.tile([128, 1], f32, tag="vd")
                    nc.tensor.matmul(vd_ps[:], wT_sb[:], adst_sb[:],
                                     start=True, stop=True)
                    nc.vector.tensor_copy(W2c[:, k, H:H + 1], vd_ps[:])

            # ---- xT via DMA transpose
            xT = bigp.tile([128, KD, NLOCP], f16)
            nc.sync.dma_start_transpose(xT[:, 0, :], xbA[:])
            nc.sync.dma_start_transpose(xT[:, 1, :], xbB[:])

            # ================= stage 1: h table + e_dst ====================
            h_sb = bigp.tile([128, NT, H], f16)
            edstloc = bigp.tile([128, NT], f32)
            with tc.tile_pool(name="s1ps", bufs=4, space="PSUM") as s1ps:
                for t in range(NT):
                    hx_ps = s1ps.tile([128, H + 1], f32, tag="hx")
                    for k in range(KD):
                        nc.tensor.matmul(hx_ps[:], xT[:, k, t * 128:(t + 1) * 128],
                                         W2c[:, k, :], start=(k == 0), stop=(k == KD - 1))
                    nc.scalar.activation(h_sb[:, t, :], hx_ps[:, 0:H], AF.Copy)
                    nc.vector.tensor_copy(edstloc[:, t:t + 1], hx_ps[:, H:H + 1])

            # write local half-tables to DRAM (rows 0..HLOC-1 and HLOC..NLOC-1)
            # h_sb[p, t, :] = node t*128+p
            NTH = HLOC // 128  # 24 full tiles per half... HLOC=3125 not mult of 128
            # HLOC = 3125 = 24*128 + 53: write per half with row-precise APs
            for half, hloc_d in ((0, hlocA_d), (1, hlocB_d)):
                base = half * HLOC
                t0, r0 = divmod(base, 128)
                rows_left = HLOC
                trow = t0
                off = r0
                dst_off = 0
                while rows_left > 0:
                    take = min(128 - off, rows_left)
                    nc.sync.dma_start(
                        bass.AP(hloc_d.tensor, dst_off * H, [[H, take], [1, H]]),
                        h_sb[off:off + take, trow, :])
                    dst_off += take
                    rows_left -= take
                    trow += 1
                    off = 0

            if bool(os.environ.get("K2_NOAG")):
                nc.sync.dma_start(hfullA_d[0:HLOC, :], hlocA_d[:])
                nc.sync.dma_start(hfullB_d[0:HLOC, :], hlocB_d[:])
            else:
                nc.gpsimd.collective_compute(
                    "AllGather", ALU.bypass, replica_groups=[list(range(NCORES))],
                    ins=[hlocA_d.opt()], outs=[hfullA_d.opt()])
                nc.gpsimd.collective_compute(
                    "AllGather", ALU.bypass, replica_groups=[list(range(NCORES))],
                    ins=[hlocB_d.opt()], outs=[hfullB_d.opt()])

            _stage = int(os.environ.get("K2_STAGE", "3"))
            _dbg = os.environ.get("K2_DBG", "")
            dbg_t = bigp.tile([128, 64], f32)
            nc.vector.memset(dbg_t[:], 0.0)
            _noag = bool(os.environ.get("K2_NOAG"))

            # ================= stage 2: edge aggregation ===================
            h0T = bigp.tile([128, NLOCP], f16)
            nc.vector.memset(h0T[:], 0.0)
            with tc.tile_pool(name="s2g", bufs=4) as s2g, \
                 tc.tile_pool(name="s2", bufs=3) as s2p, \
                 tc.tile_pool(name="s2s", bufs=6) as s2s, \
                 tc.tile_pool(name="s2ps", bufs=2, space="PSUM") as s2ps:
                # init all rotating gather buffers once so stale/pad rows are
                # finite (0) rather than arbitrary bits
                for gb in range(4):
                    g_i = s2g.tile([128, NBFmax, H], f16, tag="g")
                    nc.vector.memset(g_i[:], 0.0)
                boff = 0
                ioff = 0
                for c in range(NCH if _stage >= 2 else 0):
                    na, nb = NA[c], NB[c]
                    nbf = na + nb
                    Cc = min(C, NLOC - c * C)
                    nia, nib = NIA[c], NIB[c]

                    # gathered rows for this chunk (pad rows hold stale data;
                    # their dst_rel = -1 so S is 0 there)
                    g_c = s2g.tile([128, NBFmax, H], f16, tag="g")
                    if nia:
                        nc.gpsimd.dma_gather(
                            g_c[:, 0:na, :], hfullA_d[:],
                            srci_sb[:, ioff // 16: (ioff + nia) // 16],
                            nia, nia, H, single_packet=False)
                    if nib:
                        nc.gpsimd.dma_gather(
                            g_c[:, na:nbf, :], hfullB_d[:],
                            srci_sb[:, (ioff + nia) // 16: (ioff + nia + nib) // 16],
                            nib, nib, H, single_packet=False)
                    ioff += nia + nib

                    # e_dst broadcast grid edb[p, d] = e_dst[c*C+d]
                    edT_ps = s2ps.tile([1, 128], f32, tag="edT", bufs=1)
                    nc.tensor.matmul(edT_ps[:], edstloc[:, c:c + 1], ident[:],
                                     start=True, stop=True)
                    edrow = s2p.tile([1, 128], f32, tag="edrow")
                    nc.vector.tensor_copy(edrow[:], edT_ps[:])
                    edB_ps = s2ps.tile([128, C], f32, tag="edB", bufs=1)
                    nc.tensor.matmul(edB_ps[:], onesr[:], edrow[:],
                                     start=True, stop=True)
                    edb = s2p.tile([128, C], f32, tag="edb")
                    nc.vector.tensor_copy(edb[:], edB_ps[:])

                    # per-block: e_src, e_dst lookup, w, S
                    es = s2p.tile([128, NBFmax], f32, tag="es")
                    tl = s2p.tile([128, NBFmax], f32, tag="tl")
                    scr = s2s.tile([128, NBFmax, H], f16, tag="scr", bufs=3)
                    scr2 = s2s.tile([128, C], f32, tag="scr2", bufs=3)
                    for b in range(nbf):
                        nc.vector.scalar_tensor_tensor(
                            out=scr[:, b, :], in0=g_c[:, b, :], scalar=1.0,
                            in1=asrc_bf[:], op0=ALU.mult, op1=ALU.mult,
                            accum_out=es[:, b:b + 1])
                        nc.vector.scalar_tensor_tensor(
                            out=scr2[:], in0=iota_sb[:],
                            scalar=dstr_sb[:, boff + b: boff + b + 1],
                            in1=edb[:], op0=ALU.is_equal, op1=ALU.mult,
                            accum_out=tl[:, b:b + 1])
                    eps = s2p.tile([128, NBFmax], f32, tag="eps")
                    nc.vector.tensor_tensor(out=eps[:, 0:nbf], in0=es[:, 0:nbf],
                                            in1=tl[:, 0:nbf], op=ALU.add)
                    # clamp so pad rows can't produce inf (0*inf = NaN in S)
                    nc.vector.tensor_scalar(out=eps[:, 0:nbf], in0=eps[:, 0:nbf],
                                            scalar1=16.0, scalar2=None, op0=ALU.min)
                    lk = s2p.tile([128, NBFmax], f32, tag="lk")
                    nc.vector.tensor_scalar(out=lk[:, 0:nbf], in0=eps[:, 0:nbf],
                                            scalar1=NEG_SLOPE, scalar2=None,
                                            op0=ALU.mult)
                    nc.vector.tensor_tensor(out=lk[:, 0:nbf], in0=lk[:, 0:nbf],
                                            in1=eps[:, 0:nbf], op=ALU.max)
                    w = s2p.tile([128, NBFmax], f32, tag="w")
                    nc.scalar.activation(w[:, 0:nbf], lk[:, 0:nbf], AF.Exp, bias=neg8[:])

                    agg_ps = s2ps.tile([128, H], f32, tag="agg")
                    den_ps = s2ps.tile([128, 1], f32, tag="den")
                    for b in range(nbf):
                        S_b = s2s.tile([128, C], f16, tag="S", bufs=8)
                        nc.vector.tensor_scalar(
                            out=S_b[:], in0=iota_sb[:],
                            scalar1=dstr_sb[:, boff + b: boff + b + 1],
                            scalar2=w[:, b:b + 1],
                            op0=ALU.is_equal, op1=ALU.mult)
                        nc.tensor.matmul(agg_ps[:], S_b[:], g_c[:, b, :],
                                         start=(b == 0), stop=(b == nbf - 1))
                        nc.tensor.matmul(den_ps[:], S_b[:], onesc_bf[:],
                                         start=(b == 0), stop=(b == nbf - 1))
                    # normalize rows by denom, transpose, +b_gat, relu
                    if _dbg == "den" and c < 64:
                        nc.vector.tensor_copy(dbg_t[:, c:c + 1], den_ps[:])
                    if _dbg == "es" and c < 4:
                        nc.vector.tensor_copy(dbg_t[:, c * 16:c * 16 + min(16, nbf)],
                                              es[:, 0:min(16, nbf)])
                    if _dbg == "tl" and c < 4:
                        nc.vector.tensor_copy(dbg_t[:, c * 16:c * 16 + min(16, nbf)],
                                              tl[:, 0:min(16, nbf)])
                    if _dbg == "w" and c < 4:
                        nc.vector.tensor_copy(dbg_t[:, c * 16:c * 16 + min(16, nbf)],
                                              w[:, 0:min(16, nbf)])
                    if _dbg == "eps" and c < 4:
                        nc.vector.tensor_copy(dbg_t[:, c * 16:c * 16 + min(16, nbf)],
                                              eps[:, 0:min(16, nbf)])
                    dmax = s2p.tile([128, 1], f32, tag="dmax")
                    nc.vector.tensor_scalar(out=dmax[:], in0=den_ps[:],
                                            scalar1=1e-16, scalar2=None, op0=ALU.max)
                    rden = s2p.tile([128, 1], f32, tag="rden")
                    nc.vector.reciprocal(rden[:], dmax[:])
                    gat = s2p.tile([128, H], f16, tag="gat")
                    nc.vector.tensor_scalar(out=gat[:], in0=agg_ps[:],
                                            scalar1=rden[:], scalar2=None,
                                            op0=ALU.mult)
                    gatT_ps = s2ps.tile([128, 128], f16, tag="gatT")
                    nc.tensor.transpose(gatT_ps[:], gat[:], identb[:])
                    nc.scalar.activation(h0T[:, c * C: c * C + Cc],
                                         gatT_ps[:, 0:Cc], AF.Relu, bias=bgat_sb[:])
                    boff += nbf

            if _stage == 1:
                for t in range(NT):
                    rows = min(128, NLOC - t * 128)
                    nc.gpsimd.dma_start(out_t[t * 128: t * 128 + rows, :],
                                        h_sb[:rows, t, 0:A])
            if _stage == 2 and _dbg:
                nc.sync.dma_start(out_t[0:128, :], dbg_t[:, 0:32])
                nc.sync.dma_start(out_t[128:256, :], dbg_t[:, 32:64])
            if _stage == 2 and not _dbg:
                for t in range(NT):
                    rows = min(128, NLOC - t * 128)
                    nc.gpsimd.dma_start(
                        bass.AP(out_t, t * 128 * A, [[1, A], [A, rows]]),
                        h0T[0:A, t * 128: t * 128 + rows])

            # ================= stage 3: BN0 + MLP + softmax ================
            _s3f32 = bool(os.environ.get("K2_S3F32"))
            h1T = bigp.tile([128, NLOCP], f32 if _s3f32 else f16)
            nc.vector.memset(h1T[:, NLOC:NLOCP], 0.0)
            h2Tf = None
            if _s3f32:
                h2Tf = bigp.tile([128, NLOCP], f32, name="h2Tf")
                nc.vector.memset(h2Tf[:, NLOC:NLOCP], 0.0)
            if _stage >= 3:
             with tc.tile_pool(name="s3", bufs=2) as s3p, \
                 tc.tile_pool(name="s3ps", bufs=2, space="PSUM") as s3ps:

                def bn_fold(hT, k, Wnext_sb, bnext_sb, M):
                    s1 = s3p.tile([128, 1], f32, tag="bn1")
                    nc.vector.tensor_reduce(out=s1[:], in_=hT[:, 0:NLOC],
                                            axis=mybir.AxisListType.X, op=ALU.add)
                    sq = s3p.tile([128, NLOCP], f32, tag="bnsq")
                    s2 = s3p.tile([128, 1], f32, tag="bn2t")
                    nc.scalar.activation(sq[:, 0:NLOC], hT[:, 0:NLOC], AF.Square,
                                         accum_out=s2[:])
                    bnio = s3p.tile([128, 2], f32, tag="bnio")
                    nc.vector.tensor_copy(bnio[:, 0:1], s1[:])
                    nc.vector.tensor_copy(bnio[:, 1:2], s2[:])
                    bn_in_d = bn_in_0 if k == 0 else bn_in_1
                    bn_out_d = bn_out_0 if k == 0 else bn_out_1
                    nc.sync.dma_start(bn_in_d[:], bnio[:])
                    nc.gpsimd.collective_compute(
                        "AllReduce", ALU.add, replica_groups=[list(range(NCORES))],
                        ins=[bn_in_d.opt()], outs=[bn_out_d.opt()])
                    bnst = s3p.tile([128, 2], f32, tag="bnst")
                    nc.sync.dma_start(bnst[:], bn_out_d[:])
                    mu = s3p.tile([128, 1], f32, tag="mu")
                    nc.vector.tensor_scalar(out=mu[:], in0=bnst[:, 0:1],
                                            scalar1=1.0 / N, scalar2=None, op0=ALU.mult)
                    var = s3p.tile([128, 1], f32, tag="var")
                    nc.vector.tensor_tensor(out=var[:], in0=mu[:], in1=mu[:], op=ALU.mult)
                    nc.vector.tensor_scalar(out=var[:], in0=var[:], scalar1=-1.0,
                                            scalar2=None, op0=ALU.mult)
                    nc.vector.scalar_tensor_tensor(
                        out=var[:], in0=bnst[:, 1:2], scalar=1.0 / N, in1=var[:],
                        op0=ALU.mult, op1=ALU.add)
                    nc.vector.tensor_scalar(out=var[:], in0=var[:], scalar1=EPS,
                                            scalar2=None, op0=ALU.add)
                    rs = s3p.tile([128, 1], f32, tag="rs")
                    nc.vector.reciprocal(rs[:], var[:])
                    nc.scalar.sqrt(rs[:], rs[:])
                    bnp = bn0_sb if k == 0 else bn2_sb
                    sc = s3p.tile([128, 1], f32, tag="sc")
                    nc.vector.tensor_tensor(out=sc[:], in0=rs[:], in1=bnp[:, 0:1],
                                            op=ALU.mult)
                    u = s3p.tile([128, 1], f32, tag="u")
                    nc.vector.tensor_tensor(out=u[:], in0=mu[:], in1=sc[:], op=ALU.mult)
                    nc.vector.tensor_sub(u[:], bnp[:, 1:2], u[:])
                    Wp = s3p.tile([128, M], f32 if _s3f32 else f16,
                                  tag="wp" + str(k), name="wp_t" + str(k))
                    nc.vector.tensor_scalar(out=Wp[:], in0=Wnext_sb[:], scalar1=sc[:],
                                            scalar2=None, op0=ALU.mult)
                    brow_ps = s3ps.tile([1, M], f32, tag="brow", bufs=1)
                    nc.tensor.matmul(brow_ps[:], u[:], Wnext_sb[:], start=True, stop=True)
                    brow_sb = s3p.tile([1, M], f32, tag="brsb")
                    nc.vector.tensor_copy(brow_sb[:], brow_ps[:])
                    bcol_ps = s3ps.tile([M, 1], f32, tag="bcol", bufs=1)
                    nc.tensor.transpose(bcol_ps[:], brow_sb[:], ident[0:1, 0:1])
                    bp = s3p.tile([M, 1], f32, tag="bp" + str(k))
                    nc.vector.tensor_tensor(out=bp[:], in0=bcol_ps[:], in1=bnext_sb[:],
                                            op=ALU.add)
                    return Wp, bp

                W1p, b1p = bn_fold(h0T, 0, W1_sb, b1_sb, H)
                for s in range(0, NLOC, 512):
                    ln = min(512, NLOC - s)
                    ps = s3ps.tile([128, 512], f32, tag="mlp")
                    nc.tensor.matmul(ps[:, 0:ln], W1p[:], h0T[:, s:s + ln],
                                     start=True, stop=True)
                    nc.scalar.activation(h1T[:, s:s + ln], ps[:, 0:ln], AF.Relu,
                                         bias=b1p[:])
                h2T = h2Tf if _s3f32 else h0T
                for s in range(0, NLOC, 512):
                    ln = min(512, NLOC - s)
                    ps = s3ps.tile([128, 512], f32, tag="mlp")
                    nc.tensor.matmul(ps[:, 0:ln], W2_sb[:] if _s3f32 else W2_bf[:],
                                     h1T[:, s:s + ln],
                                     start=True, stop=True)
                    nc.scalar.activation(h2T[:, s:s + ln], ps[:, 0:ln], AF.Relu,
                                         bias=b2_sb[:])
                W3p, b3p = bn_fold(h2T, 1, W3_sb, b3_sb, A)
                actT = bigp.tile([A, NLOCP], f32)
                nc.vector.memset(actT[:], 0.0)
                for s in range(0, NLOC, 512):
                    ln = min(512, NLOC - s)
                    ps = s3ps.tile([A, 512], f32, tag="mlp")
                    nc.tensor.matmul(ps[:, 0:ln], W3p[:], h2T[:, s:s + ln],
                                     start=True, stop=True)
                    nc.vector.tensor_scalar(out=actT[0:A, s:s + ln], in0=ps[:, 0:ln],
                                            scalar1=b3p[:], scalar2=None, op0=ALU.add)
                # row softmax + output
                for t in range(NT):
                    rows = min(128, NLOC - t * 128)
                    a_sb = s3p.tile([128, A], f32, tag="asb")
                    assert A == 32
                    for sub in range(4):
                        nc.vector.transpose(
                            a_sb[32 * sub:32 * sub + 32, 0:A],
                            actT[0:A, t * 128 + 32 * sub: t * 128 + 32 * sub + 32])
                    nmax = s3p.tile([128, 1], f32, tag="nmax")
                    nc.vector.tensor_reduce(out=nmax[:], in_=a_sb[:],
                                            axis=mybir.AxisListType.X, op=ALU.max)
                    nc.vector.tensor_scalar(out=nmax[:], in0=nmax[:], scalar1=-1.0,
                                            scalar2=None, op0=ALU.mult)
                    e_sb = s3p.tile([128, A], f32, tag="esb")
                    nc.scalar.activation(e_sb[:], a_sb[:], AF.Exp, bias=nmax[:])
                    ssum = s3p.tile([128, 1], f32, tag="ssum")
                    nc.vector.tensor_reduce(out=ssum[:], in_=e_sb[:],
                                            axis=mybir.AxisListType.X, op=ALU.add)
                    rsum = s3p.tile([128, 1], f32, tag="rsum")
                    nc.vector.reciprocal(rsum[:], ssum[:])
                    o_sb = s3p.tile([128, A], f32, tag="osb")
                    nc.vector.tensor_scalar(out=o_sb[:], in0=e_sb[:], scalar1=rsum[:],
                                            scalar2=None, op0=ALU.mult)
                    nc.sync.dma_start(out_t[t * 128: t * 128 + rows, :], o_sb[:rows, :])

    nc.compile()
    return nc


# --------------------------------------------------------------------------
# public entry point
# --------------------------------------------------------------------------

def run(inputs, trace=False):
    global last_results
    x = np.asarray(inputs["x"], np.float32)
    edge_index = np.asarray(inputs["edge_index"])
    N, D = x.shape
    H = np.asarray(inputs["W"]).shape[1]
    A = np.asarray(inputs["W3"]).shape[1]
    assert N % NCORES == 0
    NLOC = N // NCORES
    NT = -(-NLOC // 128)
    NLOCP = NT * 128

    per_core, NA, NB, NIA, NIB = _prep_edges(edge_index, N, NLOC)
    TOTB = sum(NA) + sum(NB)
    TOTI = sum(NIA) + sum(NIB)

    key = (N, D, H, A, NLOC, tuple(NA), tuple(NB), tuple(NIA), tuple(NIB),
           os.environ.get("K2_STAGE"), os.environ.get("K2_NOAG"),
           os.environ.get("K2_DBG"), os.environ.get("K2_S3F32"))
    if _cache.get("key") != key:
        _cache["nc"] = _build_nc(N, D, H, A, NLOC, NA, NB, NIA, NIB)
        _cache["key"] = key
    nc = _cache["nc"]

    g = lambda k: np.ascontiguousarray(np.asarray(inputs[k], np.float32))
    common = {
        "W": g("W"),
        "adst": g("a_dst").reshape(H, 1),
        "asrc_b": np.tile(g("a_src")[None, :], (128, 1)),
        "b_gat": g("b_gat").reshape(H, 1),
        "bn0p": np.stack([g("g0"), g("beta0")], 1),
        "bn2p": np.stack([g("g2"), g("beta2")], 1),
        "W1": g("W1"), "b1": g("b1").reshape(H, 1),
        "W2": g("W2"), "b2": g("b2").reshape(H, 1),
        "W3": g("W3"), "b3": g("b3").reshape(A, 1),
        "ident": np.eye(128, dtype=np.float32),
        "iota_b": np.tile(np.arange(C, dtype=np.float32)[None, :], (128, 1)),
        "ones_row": np.ones((1, 128), np.float32),
    }
    in_maps = []
    for i in range(NCORES):
        m = dict(common)
        xs = x[i * NLOC:(i + 1) * NLOC]
        xp = np.zeros((NLOCP, D), np.float32)
        xp[:NLOC] = xs
        xb = xp.astype(np.float16)
        m["xbA"] = np.ascontiguousarray(xb[:, 0:128])
        m["xbB"] = np.ascontiguousarray(xb[:, 128:256])
        m["src_idx"] = per_core[i]["src_idx"]
        m["dst_rel"] = per_core[i]["dst_rel"]
        in_maps.append(m)

    last_results = run_bass_kernel_spmd(nc, in_maps, list(range(NCORES)),
                                        trace=trace)
    out = np.concatenate([last_results.results[i]["out"] for i in range(NCORES)], 0)
    return out


def kernel(**inputs) -> np.ndarray:
    return run(inputs, trace=False)
